# revision 41
# baseline (speedup 1.0000x reference)
"""Trainium2 Bass kernel for a single llama-style transformer layer + output head.

Model (per reference):
    h  = rms_norm(x, ln1); q,k,v = proj(h); rope(q, k)
    attn (full, non-causal) per head; x += Wo @ ctx
    h2 = rms_norm(x, ln2); x += Wdown @ (silu(Wgate h2) * (Wup h2))
    logits = x @ W_out.T + b_out            -> reshape(-1, 8, 1024)

Sharding: 8 cores, data-parallel over (batch, seq-half): core c owns batch c//2,
sequence half c%2 (1024 query tokens). Each core computes K/V for its batch's
full 2048-token sequence (small duplicate work) so no collectives are needed.

On-chip convention: activations are FEATURE-MAJOR [d, t] so the contraction
dim of every matmul is the partition dim. Weights are passed pre-transposed
(and pre-tiled where needed) from the host, with the rms-norm gains folded in.
PSUM accumulates in fp32; the residual stream stays fp32 in SBUF.

fp8 use: h/K/Q/V (and their projection weights, x64-scaled) are fp8 so the
q/k/v projections run fp8 DoubleRow (two contraction chunks per matmul);
attention probabilities are fp8 and the AV matmul runs DoubleRow over
key-chunk pairs; the MLP down-projection is DoubleRow too (gu scaled x32 via
the up-weights, Wdown x64, unscaled in the epilogue). 1/sqrt(HD) is folded
into the exp activation scale. gate/up/Wo/W_out stay bf16 for accuracy.

Scheduling: the attention phase is ScalarE-bound (exp is 1 elem/cycle/lane),
so most projection work (K/Q chunks 2-7, the second V half) is emitted as
"fill" jobs interleaved between score/exp/AV steps — keeping the PE array
dense so the HAM clock gate stays at full rate. Each pair's second AV half
is deferred into the next pair's score stream to bridge pair boundaries.
Score matmuls for the two heads of a chunk are issued back-to-back on
different PE row groups so they run concurrently.
"""

import dataclasses
import math

import numpy as np
import ml_dtypes

import concourse.bass as bass
import concourse.bacc as bacc
import concourse.tile as tile
import concourse.mybir as mybir
from concourse import bass_utils
from concourse.alu_op_type import AluOpType

BF16 = mybir.dt.bfloat16
F32 = mybir.dt.float32
FP8 = mybir.dt.float8e4
AF = mybir.ActivationFunctionType
DR = mybir.MatmulPerfMode.DoubleRow
NPBF = ml_dtypes.bfloat16
NPF8 = ml_dtypes.float8_e4m3

N_CORES = 8
GU_SCALE = 32.0
WD_SCALE = 64.0


@dataclasses.dataclass(frozen=True)
class Cfg:
    D: int = 1024      # model dim
    S: int = 2048      # full seq (per batch)
    TQ: int = 1024     # query tokens per core
    H: int = 16        # heads
    HD: int = 64       # head dim
    FF: int = 4096     # mlp intermediate
    V: int = 1024      # output head size
    NT: int = 512      # matmul moving-dim tile
    EPS: float = 1e-6
    THETA: float = 10000.0

    @property
    def CD(self):
        return self.D // 128

    @property
    def CF(self):
        return self.FF // 128

    @property
    def KT(self):
        return self.S // 128

    @property
    def HPC(self):
        return 128 // self.HD  # heads per 128-partition chunk (2)


FULL = Cfg()


def _nt_slices(total, nt):
    return [(i * nt, nt) for i in range(total // nt)]


def build_bass(cfg: Cfg):
    """Build the SPMD Bass program. Returns nc."""
    c = cfg
    nc = bacc.Bacc("TRN2", target_bir_lowering=False, debug=False,
                   num_devices=N_CORES)

    # register an eps const AP (activation() converts float biases to APs)
    _eps_t = nc.alloc_sbuf_tensor("const-eps", [128, 1], F32)
    nc.gpsimd.memset(_eps_t.ap(), c.EPS)
    nc.const_aps.aps[(F32, c.EPS)] = _eps_t.ap()

    dt = nc.dram_tensor
    x_fm = dt("x_fm", [c.D, c.S], BF16, kind="ExternalInput").ap()
    x_own = dt("x_own", [c.D, c.TQ], F32, kind="ExternalInput").ap()
    wqT = dt("wqT", [c.CD, 128, c.CD * 128], FP8, kind="ExternalInput").ap()
    wkT = dt("wkT", [c.CD, 128, c.CD * 128], FP8, kind="ExternalInput").ap()
    woT = dt("woT", [c.CD, 128, c.CD * 128], BF16, kind="ExternalInput").ap()
    wvT = dt("wvT", [c.D, c.D], FP8, kind="ExternalInput").ap()
    _W = min(512, c.FF)
    _n_fog = max(1, c.FF // 512)
    wgT = dt("wgT", [_n_fog, 128, c.CD * _W], BF16, kind="ExternalInput").ap()
    wuT = dt("wuT", [_n_fog, 128, c.CD * _W], BF16, kind="ExternalInput").ap()
    wdT = dt("wdT", [c.CD, 128, c.CF * 128], FP8, kind="ExternalInput").ap()
    woutT = dt("woutT", [c.D, c.V], BF16, kind="ExternalInput").ap()
    bias_row = dt("bias_row", [1, c.V], BF16, kind="ExternalInput").ap()
    cos_s = dt("cos_s", [128, c.S], BF16, kind="ExternalInput").ap()
    sin_s = dt("sin_s", [128, c.S], BF16, kind="ExternalInput").ap()
    shiftT = dt("shiftT", [128, 128], BF16, kind="ExternalInput").ap()
    sel = dt("sel", [c.H, c.D], BF16, kind="ExternalInput").ap()
    onesb_d = dt("onesb", [128, 128], BF16, kind="ExternalInput").ap()

    logits = dt("logits", [c.TQ, c.V], F32, kind="ExternalOutput").ap()

    with tile.TileContext(nc) as tc:
        # ---------- small whole-kernel constants ----------
        const = tc.alloc_tile_pool(name="const", bufs=1)
        ones_b = const.tile([128, 128], BF16)
        nc.sync.dma_start(ones_b[:], onesb_d[:])
        shift_sb = const.tile([128, 128], BF16)
        nc.sync.dma_start(shift_sb[:], shiftT[:])

        # ---------- right-side stack: long-lived cross-phase tensors ----------
        p_ctxn = tc.alloc_tile_pool(name="ctxn", bufs=1, side="right")
        ctxn = [p_ctxn.tile([128, c.TQ], BF16, name=f"ctxn{i}") for i in range(c.CD)]
        p_den = tc.alloc_tile_pool(name="den", bufs=1, side="right")
        den_sb = p_den.tile([c.H, c.TQ], F32)
        sel_sb = p_den.tile([c.H, c.D], BF16)
        nc.sync.dma_start(sel_sb[:], sel[:])
        bias_sb = p_den.tile([1, c.V], BF16)
        nc.sync.dma_start(bias_sb[:], bias_row[:])

        # ---------- left: K/V/Q outputs (fp8), span B -> C ----------
        p_kv = tc.alloc_tile_pool(name="kv", bufs=1)
        kr = [p_kv.tile([128, c.S], FP8, name=f"kr{i}") for i in range(c.CD)]
        # V token-major in fp8, paired key-chunks for DoubleRow AV:
        # vt2[tp][p, j, h*128+e] with j in {0,1} the key chunk 2*tp+j;
        # cols [0:HD) = V, col HD = ones (denominator trick), rest zero pad
        vt2 = [p_kv.tile([128, 2 * c.H * 128], FP8, name=f"vt2_{i}")
               for i in range(c.KT // 2)]
        p_qr = tc.alloc_tile_pool(name="qr", bufs=1)
        qr = [p_qr.tile([128, c.TQ], FP8, name=f"qr{i}") for i in range(c.CD)]

        # V weights (fp8 x64), consumed by fill jobs well into phase C
        pV_w = tc.alloc_tile_pool(name="phV_w", bufs=1)
        wv_all = pV_w.tile([128, c.CD * c.D], FP8, name="wv_all")
        for kc in range(c.CD):
            nc.sync.dma_start(wv_all[:, kc * c.D:(kc + 1) * c.D],
                              wvT[kc * 128:(kc + 1) * 128, :])
        wv_v = wv_all.rearrange("p (kc f) -> p kc f", f=c.D)

        # rope tables: Q slices them at a dynamic offset
        pB_w = tc.alloc_tile_pool(name="phB_w", bufs=1)
        cos_s_sb = pB_w.tile([128, c.S], BF16, name="cos_s_sb")
        nc.sync.dma_start(cos_s_sb[:], cos_s[:])
        sin_s_sb = pB_w.tile([128, c.S], BF16, name="sin_s_sb")
        nc.sync.dma_start(sin_s_sb[:], sin_s[:])

        # normalized activations, fp8, kc-pairs adjacent for DoubleRow
        pA = tc.alloc_tile_pool(name="phA", bufs=1)
        h_t = pA.tile([128, c.CD, c.S], FP8, name="h_t")
        p_hq = tc.alloc_tile_pool(name="hq", bufs=1)
        hq_t = p_hq.tile([128, c.CD, c.TQ], FP8, name="hq_t")

        # transients shared by phase B and the C fill jobs
        pT = tc.alloc_tile_pool(name="pT", bufs=2)
        pPS = tc.alloc_tile_pool(name="pPS", bufs=1, space="PSUM")

        # dense matmul burst at kernel start: gets the PE HAM clock gate
        # to full rate before the (sparse-PE) stats phase begins
        for i in range(24):
            pwm = pPS.tile([128, c.NT], F32, tag="pkf", name="pwm")
            nc.tensor.matmul(pwm[:], ones_b[:], cos_s_sb[:, 0:c.NT],
                             start=True, stop=True)

        # =======================================================
        # PHASE A: rms1 stats over full seq; h = x*rstd  (x resident)
        # =======================================================
        pA_x = tc.alloc_tile_pool(name="phA_x", bufs=1)
        xk = [pA_x.tile([128, c.S], BF16, name=f"xk{i}") for i in range(c.CD)]
        for cd in range(c.CD):
            nc.sync.dma_start(xk[cd][:], x_fm[cd * 128:(cd + 1) * 128, :])

        pA_t = tc.alloc_tile_pool(name="phA_t", bufs=1)
        rstd = pA_t.tile([1, c.S], BF16, name="rstd")
        pA_s = tc.alloc_tile_pool(name="phA_s", bufs=3)
        pA_ss = tc.alloc_tile_pool(name="phA_ss", bufs=1, space="PSUM")
        ss = {o: pA_ss.tile([1, c.NT], F32, name=f"ss{o}")
              for (o, n) in _nt_slices(c.S, c.NT)}
        for cd in range(c.CD):
            for (o, n) in _nt_slices(c.S, c.NT):
                sq = pA_s.tile([128, c.NT], BF16, tag="sq")
                nc.vector.tensor_tensor(sq[:], xk[cd][:, o:o + n],
                                        xk[cd][:, o:o + n], op=AluOpType.mult)
                nc.tensor.matmul(ss[o][:], ones_b[:, 0:1], sq[:],
                                 start=(cd == 0), stop=(cd == c.CD - 1))
        # rsqrt(m) = exp(-0.5 * ln(m))
        for (o, n) in _nt_slices(c.S, c.NT):
            nc.scalar.activation(rstd[:, o:o + n], ss[o][:], AF.Ln,
                                 bias=c.EPS, scale=1.0 / c.D)
        nc.scalar.activation(rstd[:], rstd[:], AF.Exp, scale=-0.5)
        pA_ss.release()

        # broadcast rstd over partitions (PE outer product) -> bf16 SBUF
        p_rb = tc.alloc_tile_pool(name="p_rb", bufs=1)
        rb_sb = p_rb.tile([128, c.S], BF16, name="rb_sb")
        pA_rb = tc.alloc_tile_pool(name="phA_rb", bufs=2, space="PSUM")
        for (o, n) in _nt_slices(c.S, c.NT):
            rbt = pA_rb.tile([128, c.NT], F32, tag="rb")
            nc.tensor.matmul(rbt[:], ones_b[0:1, :], rstd[:, o:o + n],
                             start=True, stop=True)
            nc.vector.tensor_copy(rb_sb[:, o:o + n], rbt[:])
        pA_rb.release()
        for cd in range(c.CD):
            for (o, n) in _nt_slices(c.S, c.NT):
                nc.vector.tensor_tensor(h_t[:, cd, o:o + n], xk[cd][:, o:o + n],
                                        rb_sb[:, o:o + n], op=AluOpType.mult)
        p_rb.release()
        pA_s.release()
        pA_t.release()
        pA_x.release()

        # =======================================================
        # PHASE B + C fill jobs: fp8 DoubleRow projections.
        # B runs K/Q for chunks 0-1 and the first V half; the rest is
        # interleaved into phase C as PE filler (generators below).
        # =======================================================
        halves = c.S // c.TQ
        _pid = nc.partition_id()   # register on ALL engines (PE reads it too)
        qoff = (_pid % halves) * c.TQ

        def rope_combine(raw, psk, cos_ap, sin_ap, n, dst):
            """dst = raw*cos + (S@raw)*sin, all [128, n]."""
            t1 = pT.tile([128, c.NT], BF16, tag="ropet1", name="t1")
            nc.vector.tensor_tensor(t1[:, 0:n], raw[:], cos_ap,
                                    op=AluOpType.mult)
            t2 = pT.tile([128, c.NT], BF16, tag="ropet2", name="t2")
            nc.vector.tensor_tensor(t2[:, 0:n], psk[:], sin_ap,
                                    op=AluOpType.mult)
            nc.vector.tensor_tensor(dst[:], t1[:, 0:n], t2[:, 0:n],
                                    op=AluOpType.add)

        def kq_gen(mo, wdram, dst, seqlen, dyn, ps=None):
            """Projection + rope for one output chunk of K (dyn=False)
            or Q (dyn=True, reads h at the dynamic own-half offset).
            Yields after each o-slice of work."""
            wt = pT.tile([128, c.CD * 128], FP8, tag="wfill", name=f"w{mo}")
            nc.sync.dma_start(
                wt[:], wdram[mo:mo + 1].rearrange("o p f -> (o p) f"))
            wv_ = wt.rearrange("p (kc f) -> p kc f", f=128)
            ps = ps or pPS
            for (o, n) in _nt_slices(seqlen, c.NT):
                pk = ps.tile([128, c.NT], F32, tag="pkf", name="pkf")
                for m2 in range(c.CD // 2):
                    if dyn:
                        rhs = hq_t[:, 2 * m2:2 * m2 + 2, o:o + n]
                    else:
                        rhs = h_t[:, 2 * m2:2 * m2 + 2, o:o + n]
                    nc.tensor.matmul(pk[:, 0:n], wv_[:, 2 * m2:2 * m2 + 2, :],
                                     rhs,
                                     start=(m2 == 0), stop=(m2 == c.CD // 2 - 1),
                                     perf_mode=DR)
                raw = pT.tile([128, c.NT], BF16, tag="rawf", name="rawf")
                nc.vector.tensor_scalar_mul(raw[:, 0:n], pk[:, 0:n],
                                            1.0 / WD_SCALE)
                psk = ps.tile([128, c.NT], F32, tag="pskf", name="pskf")
                nc.tensor.matmul(psk[:, 0:n], shift_sb[:], raw[:, 0:n],
                                 start=True, stop=True)
                if dyn:
                    cos_ap = cos_s_sb[:, bass.ds(qoff + o, n)]
                    sin_ap = sin_s_sb[:, bass.ds(qoff + o, n)]
                else:
                    cos_ap = cos_s_sb[:, o:o + n]
                    sin_ap = sin_s_sb[:, o:o + n]
                rope_combine(raw[:, 0:n], psk[:, 0:n], cos_ap, sin_ap, n,
                             dst[:, o:o + n])
                yield

        nh = c.NT // c.HD

        def v_gen(o2, to_range, ps=None):
            """V projection for D-half o2, one key-token chunk per step."""
            ps = ps or pPS
            for to in to_range:
                j = to % 2
                pv = ps.tile([128, c.NT], F32, tag="pkf", name=f"pv{to}")
                for m2 in range(c.CD // 2):
                    nc.tensor.matmul(
                        pv[:], h_t[:, 2 * m2:2 * m2 + 2, to * 128:(to + 1) * 128],
                        wv_v[:, 2 * m2:2 * m2 + 2, o2 * c.NT:(o2 + 1) * c.NT],
                        start=(m2 == 0), stop=(m2 == c.CD // 2 - 1),
                        perf_mode=DR)
                v4 = vt2[to // 2].rearrange("p (j h e) -> p j h e", j=2, e=128)
                nc.vector.tensor_scalar_mul(
                    v4[:, j, o2 * nh:(o2 + 1) * nh, 0:c.HD],
                    pv.rearrange("p (h e) -> p h e", e=c.HD), 1.0 / WD_SCALE)
                if o2 == 0:
                    nc.gpsimd.memset(v4[:, j, :, c.HD:c.HD + 1], 1.0)
                    nc.gpsimd.memset(v4[:, j, :, c.HD + 1:], 0.0)
                yield

        # own-half slice of h (the Q-side rms_norm equals the full-seq one)
        for cd in range(c.CD):
            nc.vector.tensor_copy(hq_t[:, cd, :], h_t[:, cd, bass.ds(qoff, c.TQ)])

        # phase B proper: K/Q chunk 0 only (the rest fills phase C), V first half
        for _ in kq_gen(0, wkT, kr[0], c.S, False):
            pass
        for _ in kq_gen(0, wqT, qr[0], c.TQ, True):
            pass
        for _ in v_gen(0, range(c.KT)):
            pass

        # =======================================================
        # PHASE C: attention. Per chunk (2 heads): two AV halves per
        # pair (o=0 while scores+exp stream, o=512 after), with the
        # remaining K/Q/V projection chunks interleaved as PE filler.
        # =======================================================
        pC_exp = tc.alloc_tile_pool(name="phC_exp", bufs=1)
        pC_ps_sc = tc.alloc_tile_pool(name="phC_sc", bufs=1, space="PSUM")
        pC_ps_av = tc.alloc_tile_pool(name="phC_av", bufs=1, space="PSUM")
        SC_SCALE = 1.0 / math.sqrt(c.HD)
        recip_bf = p_den.tile([c.H, c.TQ], BF16, name="recip_bf")
        nc.gpsimd.memset(recip_bf[:], 1.0)
        lden = p_den.tile([c.H, c.TQ], F32, name="lden")

        def norm_gen(chs):
            """1/den + ctxn normalization for finished chunks, run as
            late-pair PE/DVE filler. 1/x = exp(-ln(x)). Engine APs must
            start at partition 0, so the tail call recomputes rows 0:r1
            (idempotent: den_sb is read-only here)."""
            r1 = 2 * max(chs) + 2
            nc.scalar.activation(lden[0:r1, :], den_sb[0:r1, :], AF.Ln)
            nc.scalar.activation(recip_bf[0:r1, :], lden[0:r1, :],
                                 AF.Exp, scale=-1.0)
            yield
            for ch in chs:
                for (o, n) in _nt_slices(c.TQ, c.NT):
                    prb = pPS.tile([128, c.NT], F32, tag="pkf", name=f"prb{ch}")
                    nc.tensor.matmul(prb[:], sel_sb[:, ch * 128:(ch + 1) * 128],
                                     recip_bf[:, o:o + n], start=True, stop=True)
                    nc.vector.tensor_tensor(ctxn[ch][:, o:o + n],
                                            ctxn[ch][:, o:o + n],
                                            prb[:], op=AluOpType.mult)
                    yield

        def av_epilogue(hp, sl, i, o, n, avt):
            hh = hp * c.HPC + sl
            nc.vector.tensor_copy(ctxn[hp][sl * c.HD:sl * c.HD + c.HD, o:o + n],
                                  avt[0:c.HD, :])
            dstage = pC_exp.tile([128, c.NT], F32, tag="dstage", bufs=3,
                                 name="dstage")
            nc.vector.tensor_copy(dstage[c.HD:c.HD + 1, :],
                                  avt[c.HD:c.HD + 1, :])
            nc.sync.dma_start(den_sb[hh:hh + 1, o:o + n],
                              dstage[c.HD:c.HD + 1, :])

        fills = []

        def dummy_gen(cnt):
            """Junk DoubleRow matmuls that keep the PE HAM window busy
            through ACT-paced stretches (results never read)."""
            for i in range(cnt):
                pdm = pPS.tile([128, c.NT], F32, tag="pkf", name="pdm")
                nc.tensor.matmul(
                    pdm[:],
                    vt2[i % (c.KT // 2)].rearrange("p (j f) -> p j f", j=2)[:, :, 0:128],
                    vt2[(i + 1) % (c.KT // 2)].rearrange(
                        "p (j f) -> p j f", j=2)[:, :, 0:c.NT],
                    start=True, stop=True, perf_mode=DR)
                yield

        def fill_step(k=1):
            for _ in range(k):
                while fills:
                    try:
                        next(fills[0])
                        return
                    except StopIteration:
                        fills.pop(0)

        def make_half2(hp, et, av1):
            """AV for o=512 over all key pairs; deferred so it runs under
            the next pair's score/exp stream (keeps ACT fed and PE dense
            across the pair boundary)."""
            def emit():
                for tp in range(c.KT // 2):
                    for sl in range(c.HPC):
                        lhs = vt2[tp].rearrange(
                            "p (j f) -> p j f",
                            j=2)[:, :, (hp * c.HPC + sl) * 128:
                                 (hp * c.HPC + sl + 1) * 128]
                        nc.tensor.matmul(av1[sl][:], lhs,
                                         et[sl, tp][:, :, c.NT:2 * c.NT],
                                         start=(tp == 0),
                                         stop=(tp == c.KT // 2 - 1),
                                         perf_mode=DR)
                for sl in range(c.HPC):
                    av_epilogue(hp, sl, 1, c.NT, c.NT, av1[sl])
            return emit

        prev_half2 = None
        for hp in range(c.H // c.HPC):
            # deadline guard: everything queued in the previous pair must be
            # emitted before this pair's first score matmul reads kr/qr
            while fills:
                fill_step(1)
            # queue fill work: K/Q for the NEXT chunk, V second half early on
            if hp + 1 < c.CD:
                fills.append(kq_gen(hp + 1, wkT, kr[hp + 1], c.S, False))
                fills.append(kq_gen(hp + 1, wqT, qr[hp + 1], c.TQ, True))
            if hp < 4:
                fills.append(v_gen(1, range(hp * 4, (hp + 1) * 4)))
            if hp == 6:
                fills.append(norm_gen(range(5)))

            ch = hp
            sc = [pC_ps_sc.tile([128, c.TQ], F32, tag=f"sc{sl}", name=f"sc{hp}_{sl}")
                  for sl in range(c.HPC)]
            av0 = [pC_ps_av.tile([128, c.NT], F32, tag=f"av{sl}", name=f"av0_{hp}_{sl}")
                   for sl in range(c.HPC)]
            av1 = [pC_ps_av.tile([128, c.NT], F32, tag=f"av{sl}", name=f"av1_{hp}_{sl}")
                   for sl in range(c.HPC)]
            et = {}
            # half 1: scores + exp stream; AV for o=0 trails by one step
            for tp in range(c.KT // 2):
                if tp == 1 and prev_half2 is not None:
                    prev_half2()
                    prev_half2 = None
                for sl in range(c.HPC):
                    et[sl, tp] = pC_exp.tile([128, 2, c.TQ], FP8,
                                             tag=f"exp{sl}_{tp}",
                                             name=f"e{hp}_{sl}_{tp}",
                                             bufs=2 if tp == 0 else 1)
                for j in range(2):
                    kt = 2 * tp + j
                    for (o, n) in _nt_slices(c.TQ, c.NT):
                        for sl in range(c.HPC):
                            poff = sl * c.HD
                            nc.tensor.matmul(
                                sc[sl][:, o:o + n],
                                kr[ch][poff:poff + c.HD, kt * 128:(kt + 1) * 128],
                                qr[ch][poff:poff + c.HD, o:o + n],
                                start=True, stop=True)
                    for sl in range(c.HPC):
                        nc.scalar.activation(et[sl, tp][:, j, :], sc[sl][:],
                                             AF.Exp, scale=SC_SCALE)
                    fill_step(1)
                if tp > 0:
                    for sl in range(c.HPC):
                        lhs = vt2[tp - 1].rearrange(
                            "p (j f) -> p j f",
                            j=2)[:, :, (hp * c.HPC + sl) * 128:
                                 (hp * c.HPC + sl + 1) * 128]
                        nc.tensor.matmul(av0[sl][:], lhs,
                                         et[sl, tp - 1][:, :, 0:c.NT],
                                         start=(tp == 1), stop=False,
                                         perf_mode=DR)
                fill_step(1)
            for sl in range(c.HPC):
                lhs = vt2[c.KT // 2 - 1].rearrange(
                    "p (j f) -> p j f",
                    j=2)[:, :, (hp * c.HPC + sl) * 128:(hp * c.HPC + sl + 1) * 128]
                nc.tensor.matmul(av0[sl][:], lhs,
                                 et[sl, c.KT // 2 - 1][:, :, 0:c.NT],
                                 start=False, stop=True, perf_mode=DR)
                av_epilogue(hp, sl, 0, 0, c.NT, av0[sl])
            prev_half2 = make_half2(hp, dict(et), av1)
        prev_half2()
        while fills:
            fill_step(1)

        # tail: normalize the chunks not covered by the norm_gen filler
        for _ in norm_gen(range(5, c.CD)):
            pass
        pC_ps_av.release()
        pC_ps_sc.release()
        pPS.release()
        pC_exp.release()
        pT.release()
        p_hq.release()
        pA.release()
        pB_w.release()
        pV_w.release()
        p_qr.release()
        p_kv.release()

        n_sl_q = c.TQ // c.NT
        # E-phase gate/up weights for the first group prefetch during D
        _W2 = min(4 * 128, c.FF)
        pE_w = tc.alloc_tile_pool(name="phE_w", bufs=2)
        wg0 = pE_w.tile([128, c.CD * _W2], BF16, tag="wg", name="wg0")
        wu0 = pE_w.tile([128, c.CD * _W2], BF16, tag="wu", name="wu0")
        nc.sync.dma_start(wg0[:], wgT[0:1].rearrange("o p f -> (o p) f"))
        nc.sync.dma_start(wu0[:], wuT[0:1].rearrange("o p f -> (o p) f"))

        # =======================================================
        # PHASE D: Wo proj + residual, rms2, h2   (right-side pool)
        # =======================================================
        pD = tc.alloc_tile_pool(name="phD", bufs=1, side="right")
        xo2 = [pD.tile([128, c.TQ], F32, name=f"xo2_{i}") for i in range(c.CD)]
        h2 = [pD.tile([128, c.TQ], BF16, name=f"h2_{i}") for i in range(c.CD)]
        h3 = [pD.tile([128, c.TQ], BF16, name=f"h3_{i}") for i in range(c.CD)]
        pD_w = tc.alloc_tile_pool(name="phD_w", bufs=2)
        pD_t = tc.alloc_tile_pool(name="phD_t", bufs=3)
        pD_ps = tc.alloc_tile_pool(name="phD_ps", bufs=1, space="PSUM")
        pD_ps2 = tc.alloc_tile_pool(name="phD_ps2", bufs=2, space="PSUM")
        pD_ps1 = tc.alloc_tile_pool(name="phD_ps1", bufs=1, space="PSUM")

        ss2 = {o: pD_ps1.tile([1, c.NT], F32, name=f"ss2_{o}")
               for (o, n) in _nt_slices(c.TQ, c.NT)}
        for mo in range(c.CD):
            wo_t = pD_w.tile([128, c.CD * 128], BF16, tag="wo")
            nc.sync.dma_start(
                wo_t[:], woT[mo:mo + 1].rearrange("o p f -> (o p) f"))
            po = [pD_ps.tile([128, c.NT], F32, tag=f"po{i}", name=f"po{mo}_{i}")
                  for i in range(n_sl_q)]
            for kc in range(c.CD):
                for i, (o, n) in enumerate(_nt_slices(c.TQ, c.NT)):
                    nc.tensor.matmul(po[i][:], wo_t[:, kc * 128:(kc + 1) * 128],
                                     ctxn[kc][:, o:o + n],
                                     start=(kc == 0), stop=(kc == c.CD - 1))
            xot = pD_t.tile([128, c.TQ], F32, tag="xot")
            nc.sync.dma_start(xot[:], x_own[mo * 128:(mo + 1) * 128, :])
            for i, (o, n) in enumerate(_nt_slices(c.TQ, c.NT)):
                nc.vector.tensor_tensor(xo2[mo][:, o:o + n], xot[:, o:o + n],
                                        po[i][:], op=AluOpType.add)
                sq = pD_t.tile([128, c.NT], BF16, tag="sq2")
                nc.scalar.activation(sq[:], xo2[mo][:, o:o + n], AF.Square)
                nc.tensor.matmul(ss2[o][:], ones_b[:, 0:1], sq[:],
                                 start=(mo == 0), stop=(mo == c.CD - 1))
        rstd2 = pD_t.tile([1, c.TQ], BF16, tag="rstd2", bufs=1)
        for (o, n) in _nt_slices(c.TQ, c.NT):
            nc.scalar.activation(rstd2[:, o:o + n], ss2[o][:], AF.Ln,
                                 bias=c.EPS, scale=1.0 / c.D)
        nc.scalar.activation(rstd2[:], rstd2[:], AF.Exp, scale=-0.5)
        for (o, n) in _nt_slices(c.TQ, c.NT):
            rbt = pD_ps2.tile([128, c.NT], F32, tag="rb2")
            nc.tensor.matmul(rbt[:], ones_b[0:1, :], rstd2[:, o:o + n],
                             start=True, stop=True)
            for cd in range(c.CD):
                nc.vector.tensor_tensor(h2[cd][:, o:o + n], xo2[cd][:, o:o + n],
                                        rbt[:], op=AluOpType.mult)
                # dep-paced junk matmuls: keep the PE HAM window busy
                # through this DVE-bound stretch so phase E starts warm
                for _d in range(2):
                    pdm = pD_ps2.tile([128, c.NT], F32, tag="rb2", name="pdmD")
                    nc.tensor.matmul(pdm[:], h2[cd][:, o:o + 128],
                                     h2[cd][:, o:o + n], start=True, stop=True)

        pD_ps1.release()
        pD_ps2.release()
        pD_ps.release()
        pD_t.release()
        pD_w.release()

        # =======================================================
        # PHASE E: MLP (swiglu), t-tile outer loop
        # =======================================================
        # output-head pools (head tiles are emitted inside the E loop per
        # t-half, as soon as that half's h3 chunks are complete)
        pF = tc.alloc_tile_pool(name="phF", bufs=1)
        wout_t = pF.tile([128, c.CD * c.V], BF16)
        for kc in range(c.CD):
            nc.sync.dma_start(wout_t[:, kc * c.V:(kc + 1) * c.V],
                              woutT[kc * 128:(kc + 1) * 128, :])
        pF_t = tc.alloc_tile_pool(name="phF_t", bufs=3)
        pF_ps = tc.alloc_tile_pool(name="phF_ps", bufs=2, space="PSUM")

        def emit_head(to):
            for (o, n) in _nt_slices(c.V, c.NT):
                ph = pF_ps.tile([128, c.NT], F32, tag="ph")
                for kc in range(c.CD):
                    nc.tensor.matmul(ph[:], h3[kc][:, to * 128:(to + 1) * 128],
                                     wout_t[:, kc * c.V + o: kc * c.V + o + n],
                                     start=(kc == 0), stop=False)
                nc.tensor.matmul(ph[:], ones_b[0:1, :], bias_sb[:, o:o + n],
                                 start=False, stop=True)
                lg = pF_t.tile([128, c.NT], F32, tag="lg")
                nc.vector.tensor_copy(lg[:], ph[:])
                nc.sync.dma_start(logits[to * 128:(to + 1) * 128, o:o + n], lg[:])

        FOG = 4 * 128  # gate/up weight slice width (columns of FF)
        # gu in fp8 (x32 folded into up-weights) with fc-pairs adjacent so
        # the down projection runs fp8 DoubleRow.
        pE = tc.alloc_tile_pool(name="phE", bufs=1)
        guall = pE.tile([128, c.CF, c.NT], FP8, name="guall")
        pE_t = tc.alloc_tile_pool(name="phE_t", bufs=3)
        pE_ps = tc.alloc_tile_pool(name="phE_ps", bufs=2, space="PSUM")

        W = min(FOG, c.FF)
        n_fog = max(1, c.FF // FOG)
        fpg = c.CF // n_fog  # fo chunks per group
        for (o, n) in _nt_slices(c.TQ, c.NT):
            for fg in range(n_fog):
                if o == 0 and fg == 0:
                    wg_t, wu_t = wg0, wu0
                else:
                    wg_t = pE_w.tile([128, c.CD * W], BF16, tag="wg")
                    wu_t = pE_w.tile([128, c.CD * W], BF16, tag="wu")
                    nc.sync.dma_start(
                        wg_t[:], wgT[fg:fg + 1].rearrange("o p f -> (o p) f"))
                    nc.sync.dma_start(
                        wu_t[:], wuT[fg:fg + 1].rearrange("o p f -> (o p) f"))
                for fi in range(fpg):
                    fo = fg * fpg + fi
                    pg = pE_ps.tile([128, c.NT], F32, tag="pg")
                    pu = pE_ps.tile([128, c.NT], F32, tag="pu")
                    for kc in range(c.CD):
                        nc.tensor.matmul(
                            pg[:, 0:n],
                            wg_t[:, kc * W + fi * 128: kc * W + (fi + 1) * 128],
                            h2[kc][:, o:o + n],
                            start=(kc == 0), stop=(kc == c.CD - 1))
                    for kc in range(c.CD):
                        nc.tensor.matmul(
                            pu[:, 0:n],
                            wu_t[:, kc * W + fi * 128: kc * W + (fi + 1) * 128],
                            h2[kc][:, o:o + n],
                            start=(kc == 0), stop=(kc == c.CD - 1))
                    g = pE_t.tile([128, c.NT], BF16, tag="g")
                    nc.scalar.activation(g[:, 0:n], pg[:, 0:n], AF.Silu)
                    nc.vector.tensor_tensor(guall[:, fo, 0:n], g[:, 0:n],
                                            pu[:, 0:n], op=AluOpType.mult)
            # down proj (fp8 DoubleRow) + residual -> h3 (bf16)
            for mo in range(c.CD):
                wd_t = pE_w.tile([128, c.CF * 128], FP8, tag="wd")
                nc.sync.dma_start(
                    wd_t[:], wdT[mo:mo + 1].rearrange("o p f -> (o p) f"))
                wd_v = wd_t.rearrange("p (fc f) -> p fc f", f=128)
                pd = pE_ps.tile([128, c.NT], F32, tag="pg")
                for f2 in range(c.CF // 2):
                    nc.tensor.matmul(pd[:, 0:n],
                                     wd_v[:, 2 * f2:2 * f2 + 2, :],
                                     guall[:, 2 * f2:2 * f2 + 2, 0:n],
                                     start=(f2 == 0), stop=(f2 == c.CF // 2 - 1),
                                     perf_mode=DR)
                mt = pE_t.tile([128, c.NT], BF16, tag="mt")
                nc.scalar.mul(mt[:, 0:n], pd[:, 0:n], 1.0 / (GU_SCALE * WD_SCALE))
                nc.vector.tensor_tensor(h3[mo][:, o:o + n], xo2[mo][:, o:o + n],
                                        mt[:, 0:n], op=AluOpType.add)
            for to in range(o // 128, (o + n) // 128):
                emit_head(to)

        pE_ps.release()
        pF_ps.release()
        pE_t.release()
        pE.release()
        pF_t.release()
        pF.release()
        pE_w.release()
        pD.release()
        p_den.release()
        p_ctxn.release()
        const.release()

    nc.compile()
    return nc


# ===================== host side =====================

def _bf(a):
    return np.ascontiguousarray(np.asarray(a, dtype=np.float32)).astype(NPBF)


def make_tables(c: Cfg):
    pos = np.arange(c.S, dtype=np.float32)
    inv = 1.0 / (c.THETA ** (np.arange(0, c.HD, 2, dtype=np.float32) / c.HD))
    ang = pos[:, None] * inv[None, :]                      # [S, HD/2]
    cos = np.concatenate([np.cos(ang), np.cos(ang)], -1).T  # [HD, S]
    sin = np.concatenate([np.sin(ang), np.sin(ang)], -1).T
    sign = np.where(np.arange(c.HD) < c.HD // 2, -1.0, 1.0)[:, None].astype(np.float32)
    cos_t = _bf(np.tile(cos, (c.HPC, 1)))                  # [128, S]
    sin_t = _bf(np.tile(sin * sign, (c.HPC, 1)))

    shiftT = np.zeros((128, 128), dtype=np.float32)
    for m in range(128):
        src = m + 32 if (m % c.HD) < c.HD // 2 else m - 32
        shiftT[src, m] = 1.0
    sel = np.zeros((c.H, c.D), dtype=np.float32)
    for ch in range(c.CD):
        for m in range(128):
            sel[ch * c.HPC + m // c.HD, ch * 128 + m] = 1.0
    return cos_t, sin_t, _bf(shiftT), _bf(sel)


def tile_lhsT(wT):
    """[K, M] -> [M/128 (mo), 128 (p), K (kc*128+f)] packed lhsT rows.

    out[mo, p, kc*128+f] = wT[kc*128+p, mo*128+f] so one contiguous DMA
    yields the SBUF tile whose [:, kc*128:(kc+1)*128] slice is the
    [K=128, M=128] stationary block for contraction chunk kc.
    """
    K, M = wT.shape
    t = wT.reshape(K // 128, 128, M // 128, 128)       # [kc, p, mo, f]
    return np.ascontiguousarray(t.transpose(2, 1, 0, 3).reshape(M // 128, 128, K))


def tile_fog(wT, W):
    """[D, FF] -> [FF/W (fg), 128 (p), (D/128)*W] packed gate/up slices."""
    D, FF = wT.shape
    t = wT.reshape(D // 128, 128, FF // W, W)          # [kc, p, fg, j]
    return np.ascontiguousarray(
        t.transpose(2, 1, 0, 3).reshape(FF // W, 128, D // 128 * W))


def prep_in_maps(c: Cfg, inputs: dict, n_cores: int = N_CORES):
    x = np.asarray(inputs["chunk_hidden_states"], dtype=np.float32)  # [B,S,D]
    ln1 = np.asarray(inputs["ln1_w"], dtype=np.float32)
    ln2 = np.asarray(inputs["ln2_w"], dtype=np.float32)
    wq = np.asarray(inputs["Wq"], dtype=np.float32)
    wk = np.asarray(inputs["Wk"], dtype=np.float32)
    wv = np.asarray(inputs["Wv"], dtype=np.float32)
    wo = np.asarray(inputs["Wo"], dtype=np.float32)
    wg = np.asarray(inputs["Wgate"], dtype=np.float32)
    wu = np.asarray(inputs["Wup"], dtype=np.float32)
    wd = np.asarray(inputs["Wdown"], dtype=np.float32)
    wout = np.asarray(inputs["W_out"], dtype=np.float32)
    b_out = np.asarray(inputs["b_out"], dtype=np.float32)

    W = min(512, c.FF)
    # q/k/v projection weights in fp8 (x64 for normal-range mantissas; the
    # raw-copy unscales). 1/sqrt(HD) moves into the exp activation scale.
    wqT = tile_lhsT(((wq * ln1[None, :]).T * WD_SCALE).astype(NPF8))
    wkT = tile_lhsT(((wk * ln1[None, :]).T * WD_SCALE).astype(NPF8))
    woT = tile_lhsT(_bf(wo.T))
    wvT = ((wv * ln1[None, :]).T * WD_SCALE).astype(NPF8)
    wgT = tile_fog(_bf((wg * ln2[None, :]).T), W)
    # x32 on the up-weights scales gu into fp8 range; x64 on Wdown keeps it
    # normal-range in fp8. The product 1/2048 is unscaled in the down epilogue.
    wuT = tile_fog(_bf((wu * ln2[None, :] * GU_SCALE).T), W)
    wdT = tile_lhsT((wd.T * WD_SCALE).astype(NPF8))
    woutT = _bf(wout.T)
    bias_row = _bf(b_out[None, :])
    cos_t, sin_t, shiftT, sel = make_tables(c)
    onesb = np.ones((128, 128), dtype=np.float32).astype(NPBF)

    shared = dict(wqT=wqT, wkT=wkT, woT=woT, wvT=wvT, wgT=wgT, wuT=wuT,
                  wdT=wdT, woutT=woutT, bias_row=bias_row, cos_s=cos_t,
                  sin_s=sin_t, shiftT=shiftT, sel=sel, onesb=onesb)

    in_maps = []
    halves = c.S // c.TQ
    for core in range(n_cores):
        b, hf = core // halves, core % halves
        x_fm_f32 = np.ascontiguousarray(x[b].T)                  # [D, S]
        x_fm = x_fm_f32.astype(NPBF)
        x_own = np.ascontiguousarray(x_fm_f32[:, hf * c.TQ:(hf + 1) * c.TQ])
        m = dict(shared)
        m["x_fm"] = x_fm
        m["x_own"] = x_own
        in_maps.append(m)
    return in_maps


_NC_CACHE = {}


def _get_nc(cfg: Cfg):
    if cfg not in _NC_CACHE:
        _NC_CACHE[cfg] = build_bass(cfg)
    return _NC_CACHE[cfg]


def kernel(**inputs) -> np.ndarray:
    c = FULL
    nc = _get_nc(c)
    in_maps = prep_in_maps(c, inputs)
    res = bass_utils.run_bass_kernel_spmd(nc, in_maps, core_ids=list(range(N_CORES)))
    out = np.concatenate([res.results[i]["logits"] for i in range(N_CORES)], axis=0)
    return out.reshape(-1, 8, c.V)


# revision 43
# speedup vs baseline: 1.2052x; 1.2052x over previous
"""Trainium2 Bass kernel for a single llama-style transformer layer + output head.

Model (per reference):
    h  = rms_norm(x, ln1); q,k,v = proj(h); rope(q, k)
    attn (full, non-causal) per head; x += Wo @ ctx
    h2 = rms_norm(x, ln2); x += Wdown @ (silu(Wgate h2) * (Wup h2))
    logits = x @ W_out.T + b_out            -> reshape(-1, 8, 1024)

Sharding: 8 cores, data-parallel over (batch, seq-half): core c owns batch c//2,
sequence half c%2 (1024 query tokens). Each core computes K/V for its batch's
full 2048-token sequence (small duplicate work) so no collectives are needed.

On-chip convention: activations are FEATURE-MAJOR [d, t] so the contraction
dim of every matmul is the partition dim. Weights are passed pre-transposed
(and pre-tiled where needed) from the host, with the rms-norm gains folded in.
PSUM accumulates in fp32; the residual stream stays fp32 in SBUF.

fp8 use: h/K/Q/V (and their projection weights, x64-scaled) are fp8 so the
q/k/v projections run fp8 DoubleRow (two contraction chunks per matmul);
attention probabilities are fp8 and the AV matmul runs DoubleRow over
key-chunk pairs; the MLP down-projection is DoubleRow too (gu scaled x32 via
the up-weights, Wdown x64, unscaled in the epilogue). 1/sqrt(HD) is folded
into the exp activation scale. gate/up/Wo/W_out stay bf16 for accuracy.

Scheduling: the attention phase is ScalarE-bound (exp is 1 elem/cycle/lane),
so most projection work (K/Q chunks 2-7, the second V half) is emitted as
"fill" jobs interleaved between score/exp/AV steps — keeping the PE array
dense so the HAM clock gate stays at full rate. Each pair's second AV half
is deferred into the next pair's score stream to bridge pair boundaries.
Score matmuls for the two heads of a chunk are issued back-to-back on
different PE row groups so they run concurrently.
"""

import dataclasses
import math

import numpy as np
import ml_dtypes

import concourse.bass as bass
import concourse.bacc as bacc
import concourse.tile as tile
import concourse.mybir as mybir
from concourse import bass_utils
from concourse.alu_op_type import AluOpType

BF16 = mybir.dt.bfloat16
F32 = mybir.dt.float32
FP8 = mybir.dt.float8e4
AF = mybir.ActivationFunctionType
DR = mybir.MatmulPerfMode.DoubleRow
NPBF = ml_dtypes.bfloat16
NPF8 = ml_dtypes.float8_e4m3

N_CORES = 8
GU_SCALE = 32.0
WD_SCALE = 64.0


@dataclasses.dataclass(frozen=True)
class Cfg:
    D: int = 1024      # model dim
    S: int = 2048      # full seq (per batch)
    TQ: int = 1024     # query tokens per core
    H: int = 16        # heads
    HD: int = 64       # head dim
    FF: int = 4096     # mlp intermediate
    V: int = 1024      # output head size
    NT: int = 512      # matmul moving-dim tile
    EPS: float = 1e-6
    THETA: float = 10000.0

    @property
    def CD(self):
        return self.D // 128

    @property
    def CF(self):
        return self.FF // 128

    @property
    def KT(self):
        return self.S // 128

    @property
    def HPC(self):
        return 128 // self.HD  # heads per 128-partition chunk (2)


FULL = Cfg()


def _nt_slices(total, nt):
    return [(i * nt, nt) for i in range(total // nt)]


def build_bass(cfg: Cfg, deep_psum: bool = True):
    """Build the SPMD Bass program. Returns nc."""
    c = cfg
    nc = bacc.Bacc("TRN2", target_bir_lowering=False, debug=False,
                   num_devices=N_CORES)

    # register an eps const AP (activation() converts float biases to APs)
    _eps_t = nc.alloc_sbuf_tensor("const-eps", [128, 1], F32)
    nc.gpsimd.memset(_eps_t.ap(), c.EPS)
    nc.const_aps.aps[(F32, c.EPS)] = _eps_t.ap()

    dt = nc.dram_tensor
    x_fm = dt("x_fm", [c.D, c.S], BF16, kind="ExternalInput").ap()
    x_own = dt("x_own", [c.D, c.TQ], F32, kind="ExternalInput").ap()
    wqT = dt("wqT", [c.CD, 128, c.CD * 128], FP8, kind="ExternalInput").ap()
    wkT = dt("wkT", [c.CD, 128, c.CD * 128], FP8, kind="ExternalInput").ap()
    woT = dt("woT", [c.CD, 128, c.CD * 128], BF16, kind="ExternalInput").ap()
    wvT = dt("wvT", [c.D, c.D], FP8, kind="ExternalInput").ap()
    _W = min(512, c.FF)
    _n_fog = max(1, c.FF // 512)
    wgT = dt("wgT", [_n_fog, 128, c.CD * _W], BF16, kind="ExternalInput").ap()
    wuT = dt("wuT", [_n_fog, 128, c.CD * _W], BF16, kind="ExternalInput").ap()
    wdT = dt("wdT", [c.CD, 128, c.CF * 128], FP8, kind="ExternalInput").ap()
    woutT = dt("woutT", [c.D, c.V], BF16, kind="ExternalInput").ap()
    bias_row = dt("bias_row", [1, c.V], BF16, kind="ExternalInput").ap()
    cos_s = dt("cos_s", [128, c.S], BF16, kind="ExternalInput").ap()
    sin_s = dt("sin_s", [128, c.S], BF16, kind="ExternalInput").ap()
    shiftT = dt("shiftT", [128, 128], BF16, kind="ExternalInput").ap()
    sel = dt("sel", [c.H, c.D], BF16, kind="ExternalInput").ap()
    onesb_d = dt("onesb", [128, 128], BF16, kind="ExternalInput").ap()

    logits = dt("logits", [c.TQ, c.V], F32, kind="ExternalOutput").ap()

    with tile.TileContext(nc) as tc:
        # ---------- small whole-kernel constants ----------
        const = tc.alloc_tile_pool(name="const", bufs=1)
        ones_b = const.tile([128, 128], BF16)
        nc.sync.dma_start(ones_b[:], onesb_d[:])
        shift_sb = const.tile([128, 128], BF16)
        nc.sync.dma_start(shift_sb[:], shiftT[:])

        # ---------- right-side stack: long-lived cross-phase tensors ----------
        p_ctxn = tc.alloc_tile_pool(name="ctxn", bufs=1, side="right")
        ctxn = [p_ctxn.tile([128, c.TQ], BF16, name=f"ctxn{i}") for i in range(c.CD)]
        p_den = tc.alloc_tile_pool(name="den", bufs=1, side="right")
        den_sb = p_den.tile([c.H, c.TQ], F32)
        sel_sb = p_den.tile([c.H, c.D], BF16)
        nc.sync.dma_start(sel_sb[:], sel[:])
        bias_sb = p_den.tile([1, c.V], BF16)
        nc.sync.dma_start(bias_sb[:], bias_row[:])

        # ---------- left: K/V/Q outputs (fp8), span B -> C ----------
        p_kv = tc.alloc_tile_pool(name="kv", bufs=1)
        kr = [p_kv.tile([128, c.S], FP8, name=f"kr{i}") for i in range(c.CD)]
        # V token-major in fp8, paired key-chunks for DoubleRow AV:
        # vt2[tp][p, j, h*128+e] with j in {0,1} the key chunk 2*tp+j;
        # cols [0:HD) = V, col HD = ones (denominator trick), rest zero pad
        vt2 = [p_kv.tile([128, 2 * c.H * 128], FP8, name=f"vt2_{i}")
               for i in range(c.KT // 2)]
        p_qr = tc.alloc_tile_pool(name="qr", bufs=1)
        qr = [p_qr.tile([128, c.TQ], FP8, name=f"qr{i}") for i in range(c.CD)]

        # V weights (fp8 x64), consumed by fill jobs well into phase C
        pV_w = tc.alloc_tile_pool(name="phV_w", bufs=1)
        wv_all = pV_w.tile([128, c.CD * c.D], FP8, name="wv_all")
        for kc in range(c.CD):
            nc.sync.dma_start(wv_all[:, kc * c.D:(kc + 1) * c.D],
                              wvT[kc * 128:(kc + 1) * 128, :])
        wv_v = wv_all.rearrange("p (kc f) -> p kc f", f=c.D)

        # rope tables: Q slices them at a dynamic offset
        pB_w = tc.alloc_tile_pool(name="phB_w", bufs=1)
        cos_s_sb = pB_w.tile([128, c.S], BF16, name="cos_s_sb")
        nc.sync.dma_start(cos_s_sb[:], cos_s[:])
        sin_s_sb = pB_w.tile([128, c.S], BF16, name="sin_s_sb")
        nc.sync.dma_start(sin_s_sb[:], sin_s[:])

        # normalized activations, fp8, kc-pairs adjacent for DoubleRow
        pA = tc.alloc_tile_pool(name="phA", bufs=1)
        h_t = pA.tile([128, c.CD, c.S], FP8, name="h_t")
        p_hq = tc.alloc_tile_pool(name="hq", bufs=1)
        hq_t = p_hq.tile([128, c.CD, c.TQ], FP8, name="hq_t")

        # transients shared by phase B and the C fill jobs
        pT = tc.alloc_tile_pool(name="pT", bufs=2)
        pPS = tc.alloc_tile_pool(name="pPS", bufs=1, space="PSUM")

        # dense matmul burst at kernel start: gets the PE HAM clock gate
        # to full rate before the (sparse-PE) stats phase begins
        for i in range(24):
            pwm = pPS.tile([128, c.NT], F32, tag="pkf", name="pwm")
            nc.tensor.matmul(pwm[:], ones_b[:], cos_s_sb[:, 0:c.NT],
                             start=True, stop=True)

        # =======================================================
        # PHASE A: rms1 stats over full seq; h = x*rstd  (x resident)
        # =======================================================
        pA_x = tc.alloc_tile_pool(name="phA_x", bufs=1)
        xk = [pA_x.tile([128, c.S], BF16, name=f"xk{i}") for i in range(c.CD)]
        for cd in range(c.CD):
            nc.sync.dma_start(xk[cd][:], x_fm[cd * 128:(cd + 1) * 128, :])

        pA_t = tc.alloc_tile_pool(name="phA_t", bufs=1)
        rstd = pA_t.tile([1, c.S], BF16, name="rstd")
        pA_s = tc.alloc_tile_pool(name="phA_s", bufs=3)
        pA_ss = tc.alloc_tile_pool(name="phA_ss", bufs=1, space="PSUM")
        ss = {o: pA_ss.tile([1, c.NT], F32, name=f"ss{o}")
              for (o, n) in _nt_slices(c.S, c.NT)}
        for cd in range(c.CD):
            for (o, n) in _nt_slices(c.S, c.NT):
                sq = pA_s.tile([128, c.NT], BF16, tag="sq")
                nc.vector.tensor_tensor(sq[:], xk[cd][:, o:o + n],
                                        xk[cd][:, o:o + n], op=AluOpType.mult)
                nc.tensor.matmul(ss[o][:], ones_b[:, 0:1], sq[:],
                                 start=(cd == 0), stop=(cd == c.CD - 1))
        # rsqrt(m) = exp(-0.5 * ln(m))
        for (o, n) in _nt_slices(c.S, c.NT):
            nc.scalar.activation(rstd[:, o:o + n], ss[o][:], AF.Ln,
                                 bias=c.EPS, scale=1.0 / c.D)
        nc.scalar.activation(rstd[:], rstd[:], AF.Exp, scale=-0.5)
        pA_ss.release()

        # broadcast rstd over partitions (PE outer product) -> bf16 SBUF
        p_rb = tc.alloc_tile_pool(name="p_rb", bufs=1)
        rb_sb = p_rb.tile([128, c.S], BF16, name="rb_sb")
        pA_rb = tc.alloc_tile_pool(name="phA_rb", bufs=2, space="PSUM")
        for (o, n) in _nt_slices(c.S, c.NT):
            rbt = pA_rb.tile([128, c.NT], F32, tag="rb")
            nc.tensor.matmul(rbt[:], ones_b[0:1, :], rstd[:, o:o + n],
                             start=True, stop=True)
            nc.vector.tensor_copy(rb_sb[:, o:o + n], rbt[:])
        pA_rb.release()
        for cd in range(c.CD):
            for (o, n) in _nt_slices(c.S, c.NT):
                nc.vector.tensor_tensor(h_t[:, cd, o:o + n], xk[cd][:, o:o + n],
                                        rb_sb[:, o:o + n], op=AluOpType.mult)
        p_rb.release()
        pA_s.release()
        pA_t.release()
        pA_x.release()

        # =======================================================
        # PHASE B + C fill jobs: fp8 DoubleRow projections.
        # B runs K/Q for chunks 0-1 and the first V half; the rest is
        # interleaved into phase C as PE filler (generators below).
        # =======================================================
        halves = c.S // c.TQ
        _pid = nc.partition_id()   # register on ALL engines (PE reads it too)
        qoff = (_pid % halves) * c.TQ

        def rope_combine(raw, psk, cos_ap, sin_ap, n, dst):
            """dst = raw*cos + (S@raw)*sin, all [128, n]."""
            t1 = pT.tile([128, c.NT], BF16, tag="ropet1", name="t1")
            nc.vector.tensor_tensor(t1[:, 0:n], raw[:], cos_ap,
                                    op=AluOpType.mult)
            t2 = pT.tile([128, c.NT], BF16, tag="ropet2", name="t2")
            nc.vector.tensor_tensor(t2[:, 0:n], psk[:], sin_ap,
                                    op=AluOpType.mult)
            nc.vector.tensor_tensor(dst[:], t1[:, 0:n], t2[:, 0:n],
                                    op=AluOpType.add)

        def kq_gen(mo, wdram, dst, seqlen, dyn, ps=None):
            """Projection + rope for one output chunk of K (dyn=False)
            or Q (dyn=True, reads h at the dynamic own-half offset).
            Yields after each o-slice of work."""
            wt = pT.tile([128, c.CD * 128], FP8, tag="wfill", name=f"w{mo}")
            nc.sync.dma_start(
                wt[:], wdram[mo:mo + 1].rearrange("o p f -> (o p) f"))
            wv_ = wt.rearrange("p (kc f) -> p kc f", f=128)
            ps = ps or pPS
            for (o, n) in _nt_slices(seqlen, c.NT):
                pk = ps.tile([128, c.NT], F32, tag="pkf", name="pkf")
                for m2 in range(c.CD // 2):
                    if dyn:
                        rhs = hq_t[:, 2 * m2:2 * m2 + 2, o:o + n]
                    else:
                        rhs = h_t[:, 2 * m2:2 * m2 + 2, o:o + n]
                    nc.tensor.matmul(pk[:, 0:n], wv_[:, 2 * m2:2 * m2 + 2, :],
                                     rhs,
                                     start=(m2 == 0), stop=(m2 == c.CD // 2 - 1),
                                     perf_mode=DR)
                raw = pT.tile([128, c.NT], BF16, tag="rawf", name="rawf")
                nc.vector.tensor_scalar_mul(raw[:, 0:n], pk[:, 0:n],
                                            1.0 / WD_SCALE)
                psk = ps.tile([128, c.NT], F32, tag="pskf", name="pskf")
                nc.tensor.matmul(psk[:, 0:n], shift_sb[:], raw[:, 0:n],
                                 start=True, stop=True)
                if dyn:
                    cos_ap = cos_s_sb[:, bass.ds(qoff + o, n)]
                    sin_ap = sin_s_sb[:, bass.ds(qoff + o, n)]
                else:
                    cos_ap = cos_s_sb[:, o:o + n]
                    sin_ap = sin_s_sb[:, o:o + n]
                rope_combine(raw[:, 0:n], psk[:, 0:n], cos_ap, sin_ap, n,
                             dst[:, o:o + n])
                yield

        nh = c.NT // c.HD

        def v_gen(o2, to_range, ps=None):
            """V projection for D-half o2, one key-token chunk per step."""
            ps = ps or pPS
            for to in to_range:
                j = to % 2
                pv = ps.tile([128, c.NT], F32, tag="pkf", name=f"pv{to}")
                for m2 in range(c.CD // 2):
                    nc.tensor.matmul(
                        pv[:], h_t[:, 2 * m2:2 * m2 + 2, to * 128:(to + 1) * 128],
                        wv_v[:, 2 * m2:2 * m2 + 2, o2 * c.NT:(o2 + 1) * c.NT],
                        start=(m2 == 0), stop=(m2 == c.CD // 2 - 1),
                        perf_mode=DR)
                v4 = vt2[to // 2].rearrange("p (j h e) -> p j h e", j=2, e=128)
                nc.vector.tensor_scalar_mul(
                    v4[:, j, o2 * nh:(o2 + 1) * nh, 0:c.HD],
                    pv.rearrange("p (h e) -> p h e", e=c.HD), 1.0 / WD_SCALE)
                if o2 == 0:
                    nc.gpsimd.memset(v4[:, j, :, c.HD:c.HD + 1], 1.0)
                    nc.gpsimd.memset(v4[:, j, :, c.HD + 1:], 0.0)
                yield

        # own-half slice of h (the Q-side rms_norm equals the full-seq one)
        for cd in range(c.CD):
            nc.vector.tensor_copy(hq_t[:, cd, :], h_t[:, cd, bass.ds(qoff, c.TQ)])

        # phase B proper: K/Q chunk 0 only (the rest fills phase C), V first half
        pB2 = (tc.alloc_tile_pool(name="phB2_ps", bufs=2, space="PSUM")
               if deep_psum else None)
        for _ in kq_gen(0, wkT, kr[0], c.S, False, ps=pB2):
            pass
        for _ in kq_gen(0, wqT, qr[0], c.TQ, True, ps=pB2):
            pass
        for _ in v_gen(0, range(c.KT), ps=pB2):
            pass
        if pB2 is not None:
            pB2.release()

        # =======================================================
        # PHASE C: attention. Per chunk (2 heads): two AV halves per
        # pair (o=0 while scores+exp stream, o=512 after), with the
        # remaining K/Q/V projection chunks interleaved as PE filler.
        # =======================================================
        pC_exp = tc.alloc_tile_pool(name="phC_exp", bufs=1)
        pC_ps_sc = tc.alloc_tile_pool(name="phC_sc", bufs=1, space="PSUM")
        pC_ps_av = tc.alloc_tile_pool(name="phC_av", bufs=1, space="PSUM")
        SC_SCALE = 1.0 / math.sqrt(c.HD)
        recip_bf = p_den.tile([c.H, c.TQ], BF16, name="recip_bf")
        nc.gpsimd.memset(recip_bf[:], 1.0)
        lden = p_den.tile([c.H, c.TQ], F32, name="lden")

        def norm_gen(chs):
            """1/den + ctxn normalization for finished chunks, run as
            late-pair PE/DVE filler. 1/x = exp(-ln(x)). Engine APs must
            start at partition 0, so the tail call recomputes rows 0:r1
            (idempotent: den_sb is read-only here)."""
            r1 = 2 * max(chs) + 2
            nc.scalar.activation(lden[0:r1, :], den_sb[0:r1, :], AF.Ln)
            nc.scalar.activation(recip_bf[0:r1, :], lden[0:r1, :],
                                 AF.Exp, scale=-1.0)
            yield
            for ch in chs:
                for (o, n) in _nt_slices(c.TQ, c.NT):
                    prb = pPS.tile([128, c.NT], F32, tag="pkf", name=f"prb{ch}")
                    nc.tensor.matmul(prb[:], sel_sb[:, ch * 128:(ch + 1) * 128],
                                     recip_bf[:, o:o + n], start=True, stop=True)
                    nc.vector.tensor_tensor(ctxn[ch][:, o:o + n],
                                            ctxn[ch][:, o:o + n],
                                            prb[:], op=AluOpType.mult)
                    yield

        def av_epilogue(hp, sl, i, o, n, avt):
            hh = hp * c.HPC + sl
            nc.vector.tensor_copy(ctxn[hp][sl * c.HD:sl * c.HD + c.HD, o:o + n],
                                  avt[0:c.HD, :])
            dstage = pC_exp.tile([128, c.NT], F32, tag="dstage", bufs=3,
                                 name="dstage")
            nc.vector.tensor_copy(dstage[c.HD:c.HD + 1, :],
                                  avt[c.HD:c.HD + 1, :])
            nc.sync.dma_start(den_sb[hh:hh + 1, o:o + n],
                              dstage[c.HD:c.HD + 1, :])

        fills = []

        def dummy_gen(cnt):
            """Junk DoubleRow matmuls that keep the PE HAM window busy
            through ACT-paced stretches (results never read)."""
            for i in range(cnt):
                pdm = pPS.tile([128, c.NT], F32, tag="pkf", name="pdm")
                nc.tensor.matmul(
                    pdm[:],
                    vt2[i % (c.KT // 2)].rearrange("p (j f) -> p j f", j=2)[:, :, 0:128],
                    vt2[(i + 1) % (c.KT // 2)].rearrange(
                        "p (j f) -> p j f", j=2)[:, :, 0:c.NT],
                    start=True, stop=True, perf_mode=DR)
                yield

        def fill_step(k=1):
            for _ in range(k):
                while fills:
                    try:
                        next(fills[0])
                        return
                    except StopIteration:
                        fills.pop(0)

        def make_half2(hp, et, av1):
            """AV for o=512 over all key pairs; deferred so it runs under
            the next pair's score/exp stream (keeps ACT fed and PE dense
            across the pair boundary)."""
            def emit():
                for tp in range(c.KT // 2):
                    for sl in range(c.HPC):
                        lhs = vt2[tp].rearrange(
                            "p (j f) -> p j f",
                            j=2)[:, :, (hp * c.HPC + sl) * 128:
                                 (hp * c.HPC + sl + 1) * 128]
                        nc.tensor.matmul(av1[sl][:], lhs,
                                         et[sl, tp][:, :, c.NT:2 * c.NT],
                                         start=(tp == 0),
                                         stop=(tp == c.KT // 2 - 1),
                                         perf_mode=DR)
                for sl in range(c.HPC):
                    av_epilogue(hp, sl, 1, c.NT, c.NT, av1[sl])
            return emit

        prev_half2 = None
        for hp in range(c.H // c.HPC):
            # deadline guard: everything queued in the previous pair must be
            # emitted before this pair's first score matmul reads kr/qr
            while fills:
                fill_step(1)
            # queue fill work: K/Q for the NEXT chunk, V second half early on
            if hp + 1 < c.CD:
                fills.append(kq_gen(hp + 1, wkT, kr[hp + 1], c.S, False))
                fills.append(kq_gen(hp + 1, wqT, qr[hp + 1], c.TQ, True))
            if hp < 4:
                fills.append(v_gen(1, range(hp * 4, (hp + 1) * 4)))
            if hp == 6:
                fills.append(norm_gen(range(5)))

            ch = hp
            sc = [pC_ps_sc.tile([128, c.TQ], F32, tag=f"sc{sl}", name=f"sc{hp}_{sl}")
                  for sl in range(c.HPC)]
            av0 = [pC_ps_av.tile([128, c.NT], F32, tag=f"av{sl}", name=f"av0_{hp}_{sl}")
                   for sl in range(c.HPC)]
            av1 = [pC_ps_av.tile([128, c.NT], F32, tag=f"av{sl}", name=f"av1_{hp}_{sl}")
                   for sl in range(c.HPC)]
            et = {}
            # half 1: scores + exp stream; AV for o=0 trails by one step
            for tp in range(c.KT // 2):
                if tp == 1 and prev_half2 is not None:
                    prev_half2()
                    prev_half2 = None
                for sl in range(c.HPC):
                    et[sl, tp] = pC_exp.tile([128, 2, c.TQ], FP8,
                                             tag=f"exp{sl}_{tp}",
                                             name=f"e{hp}_{sl}_{tp}",
                                             bufs=2 if tp == 0 else 1)
                for j in range(2):
                    kt = 2 * tp + j
                    for (o, n) in _nt_slices(c.TQ, c.NT):
                        for sl in range(c.HPC):
                            poff = sl * c.HD
                            nc.tensor.matmul(
                                sc[sl][:, o:o + n],
                                kr[ch][poff:poff + c.HD, kt * 128:(kt + 1) * 128],
                                qr[ch][poff:poff + c.HD, o:o + n],
                                start=True, stop=True)
                    for sl in range(c.HPC):
                        nc.scalar.activation(et[sl, tp][:, j, :], sc[sl][:],
                                             AF.Exp, scale=SC_SCALE)
                    fill_step(1)
                if tp > 0:
                    for sl in range(c.HPC):
                        lhs = vt2[tp - 1].rearrange(
                            "p (j f) -> p j f",
                            j=2)[:, :, (hp * c.HPC + sl) * 128:
                                 (hp * c.HPC + sl + 1) * 128]
                        nc.tensor.matmul(av0[sl][:], lhs,
                                         et[sl, tp - 1][:, :, 0:c.NT],
                                         start=(tp == 1), stop=False,
                                         perf_mode=DR)
                fill_step(1)
            for sl in range(c.HPC):
                lhs = vt2[c.KT // 2 - 1].rearrange(
                    "p (j f) -> p j f",
                    j=2)[:, :, (hp * c.HPC + sl) * 128:(hp * c.HPC + sl + 1) * 128]
                nc.tensor.matmul(av0[sl][:], lhs,
                                 et[sl, c.KT // 2 - 1][:, :, 0:c.NT],
                                 start=False, stop=True, perf_mode=DR)
                av_epilogue(hp, sl, 0, 0, c.NT, av0[sl])
            prev_half2 = make_half2(hp, dict(et), av1)
        prev_half2()
        while fills:
            fill_step(1)

        # tail: normalize the chunks not covered by the norm_gen filler
        for _ in norm_gen(range(5, c.CD)):
            pass
        pC_ps_av.release()
        pC_ps_sc.release()
        pPS.release()
        pC_exp.release()
        pT.release()
        p_hq.release()
        pA.release()
        pB_w.release()
        pV_w.release()
        p_qr.release()
        p_kv.release()

        n_sl_q = c.TQ // c.NT
        # E-phase gate/up weights for the first group prefetch during D
        _W2 = min(4 * 128, c.FF)
        pE_w = tc.alloc_tile_pool(name="phE_w", bufs=2)
        wg0 = pE_w.tile([128, c.CD * _W2], BF16, tag="wg", name="wg0")
        wu0 = pE_w.tile([128, c.CD * _W2], BF16, tag="wu", name="wu0")
        nc.sync.dma_start(wg0[:], wgT[0:1].rearrange("o p f -> (o p) f"))
        nc.sync.dma_start(wu0[:], wuT[0:1].rearrange("o p f -> (o p) f"))

        # =======================================================
        # PHASE D: Wo proj + residual, rms2, h2   (right-side pool)
        # =======================================================
        pD = tc.alloc_tile_pool(name="phD", bufs=1, side="right")
        xo2 = [pD.tile([128, c.TQ], F32, name=f"xo2_{i}") for i in range(c.CD)]
        h2 = [pD.tile([128, c.TQ], BF16, name=f"h2_{i}") for i in range(c.CD)]
        h3 = [pD.tile([128, c.TQ], BF16, name=f"h3_{i}") for i in range(c.CD)]
        pD_w = tc.alloc_tile_pool(name="phD_w", bufs=2)
        pD_t = tc.alloc_tile_pool(name="phD_t", bufs=3)
        pD_ps = tc.alloc_tile_pool(name="phD_ps", bufs=2 if deep_psum else 1,
                                   space="PSUM")
        pD_ps2 = tc.alloc_tile_pool(name="phD_ps2", bufs=2, space="PSUM")
        pD_ps1 = tc.alloc_tile_pool(name="phD_ps1", bufs=1, space="PSUM")

        ss2 = {o: pD_ps1.tile([1, c.NT], F32, name=f"ss2_{o}")
               for (o, n) in _nt_slices(c.TQ, c.NT)}
        for mo in range(c.CD):
            wo_t = pD_w.tile([128, c.CD * 128], BF16, tag="wo")
            nc.sync.dma_start(
                wo_t[:], woT[mo:mo + 1].rearrange("o p f -> (o p) f"))
            po = [pD_ps.tile([128, c.NT], F32, tag=f"po{i}", name=f"po{mo}_{i}")
                  for i in range(n_sl_q)]
            for kc in range(c.CD):
                for i, (o, n) in enumerate(_nt_slices(c.TQ, c.NT)):
                    nc.tensor.matmul(po[i][:], wo_t[:, kc * 128:(kc + 1) * 128],
                                     ctxn[kc][:, o:o + n],
                                     start=(kc == 0), stop=(kc == c.CD - 1))
            xot = pD_t.tile([128, c.TQ], F32, tag="xot")
            nc.sync.dma_start(xot[:], x_own[mo * 128:(mo + 1) * 128, :])
            for i, (o, n) in enumerate(_nt_slices(c.TQ, c.NT)):
                nc.vector.tensor_tensor(xo2[mo][:, o:o + n], xot[:, o:o + n],
                                        po[i][:], op=AluOpType.add)
                sq = pD_t.tile([128, c.NT], BF16, tag="sq2")
                nc.scalar.activation(sq[:], xo2[mo][:, o:o + n], AF.Square)
                nc.tensor.matmul(ss2[o][:], ones_b[:, 0:1], sq[:],
                                 start=(mo == 0), stop=(mo == c.CD - 1))
        rstd2 = pD_t.tile([1, c.TQ], BF16, tag="rstd2", bufs=1)
        for (o, n) in _nt_slices(c.TQ, c.NT):
            nc.scalar.activation(rstd2[:, o:o + n], ss2[o][:], AF.Ln,
                                 bias=c.EPS, scale=1.0 / c.D)
        nc.scalar.activation(rstd2[:], rstd2[:], AF.Exp, scale=-0.5)
        for (o, n) in _nt_slices(c.TQ, c.NT):
            rbt = pD_ps2.tile([128, c.NT], F32, tag="rb2")
            nc.tensor.matmul(rbt[:], ones_b[0:1, :], rstd2[:, o:o + n],
                             start=True, stop=True)
            for cd in range(c.CD):
                nc.vector.tensor_tensor(h2[cd][:, o:o + n], xo2[cd][:, o:o + n],
                                        rbt[:], op=AluOpType.mult)
                # dep-paced junk matmuls: keep the PE HAM window busy
                # through this DVE-bound stretch so phase E starts warm
                for _d in range(2):
                    pdm = pD_ps2.tile([128, c.NT], F32, tag="rb2", name="pdmD")
                    nc.tensor.matmul(pdm[:], h2[cd][:, o:o + 128],
                                     h2[cd][:, o:o + n], start=True, stop=True)

        pD_ps1.release()
        pD_ps2.release()
        pD_ps.release()
        pD_t.release()
        pD_w.release()

        # =======================================================
        # PHASE E: MLP (swiglu), t-tile outer loop
        # =======================================================
        # output-head pools (head tiles are emitted inside the E loop per
        # t-half, as soon as that half's h3 chunks are complete)
        pF = tc.alloc_tile_pool(name="phF", bufs=1)
        wout_t = pF.tile([128, c.CD * c.V], BF16)
        for kc in range(c.CD):
            nc.sync.dma_start(wout_t[:, kc * c.V:(kc + 1) * c.V],
                              woutT[kc * 128:(kc + 1) * 128, :])
        pF_t = tc.alloc_tile_pool(name="phF_t", bufs=3)
        pF_ps = tc.alloc_tile_pool(name="phF_ps", bufs=2, space="PSUM")

        def emit_head(to):
            for (o, n) in _nt_slices(c.V, c.NT):
                ph = pF_ps.tile([128, c.NT], F32, tag="ph")
                for kc in range(c.CD):
                    nc.tensor.matmul(ph[:], h3[kc][:, to * 128:(to + 1) * 128],
                                     wout_t[:, kc * c.V + o: kc * c.V + o + n],
                                     start=(kc == 0), stop=False)
                nc.tensor.matmul(ph[:], ones_b[0:1, :], bias_sb[:, o:o + n],
                                 start=False, stop=True)
                lg = pF_t.tile([128, c.NT], F32, tag="lg")
                nc.vector.tensor_copy(lg[:], ph[:])
                nc.sync.dma_start(logits[to * 128:(to + 1) * 128, o:o + n], lg[:])

        FOG = 4 * 128  # gate/up weight slice width (columns of FF)
        # gu in fp8 (x32 folded into up-weights) with fc-pairs adjacent so
        # the down projection runs fp8 DoubleRow.
        pE = tc.alloc_tile_pool(name="phE", bufs=1)
        guall = pE.tile([128, c.CF, c.NT], FP8, name="guall")
        pE_t = tc.alloc_tile_pool(name="phE_t", bufs=3)
        pE_ps = tc.alloc_tile_pool(name="phE_ps", bufs=2, space="PSUM")

        W = min(FOG, c.FF)
        n_fog = max(1, c.FF // FOG)
        fpg = c.CF // n_fog  # fo chunks per group
        for (o, n) in _nt_slices(c.TQ, c.NT):
            for fg in range(n_fog):
                if o == 0 and fg == 0:
                    wg_t, wu_t = wg0, wu0
                else:
                    wg_t = pE_w.tile([128, c.CD * W], BF16, tag="wg")
                    wu_t = pE_w.tile([128, c.CD * W], BF16, tag="wu")
                    nc.sync.dma_start(
                        wg_t[:], wgT[fg:fg + 1].rearrange("o p f -> (o p) f"))
                    nc.sync.dma_start(
                        wu_t[:], wuT[fg:fg + 1].rearrange("o p f -> (o p) f"))
                for fi in range(fpg):
                    fo = fg * fpg + fi
                    pg = pE_ps.tile([128, c.NT], F32, tag="pg")
                    pu = pE_ps.tile([128, c.NT], F32, tag="pu")
                    for kc in range(c.CD):
                        nc.tensor.matmul(
                            pg[:, 0:n],
                            wg_t[:, kc * W + fi * 128: kc * W + (fi + 1) * 128],
                            h2[kc][:, o:o + n],
                            start=(kc == 0), stop=(kc == c.CD - 1))
                    for kc in range(c.CD):
                        nc.tensor.matmul(
                            pu[:, 0:n],
                            wu_t[:, kc * W + fi * 128: kc * W + (fi + 1) * 128],
                            h2[kc][:, o:o + n],
                            start=(kc == 0), stop=(kc == c.CD - 1))
                    g = pE_t.tile([128, c.NT], BF16, tag="g")
                    nc.scalar.activation(g[:, 0:n], pg[:, 0:n], AF.Silu)
                    nc.vector.tensor_tensor(guall[:, fo, 0:n], g[:, 0:n],
                                            pu[:, 0:n], op=AluOpType.mult)
            # down proj (fp8 DoubleRow) + residual -> h3 (bf16)
            for mo in range(c.CD):
                wd_t = pE_w.tile([128, c.CF * 128], FP8, tag="wd")
                nc.sync.dma_start(
                    wd_t[:], wdT[mo:mo + 1].rearrange("o p f -> (o p) f"))
                wd_v = wd_t.rearrange("p (fc f) -> p fc f", f=128)
                pd = pE_ps.tile([128, c.NT], F32, tag="pg")
                for f2 in range(c.CF // 2):
                    nc.tensor.matmul(pd[:, 0:n],
                                     wd_v[:, 2 * f2:2 * f2 + 2, :],
                                     guall[:, 2 * f2:2 * f2 + 2, 0:n],
                                     start=(f2 == 0), stop=(f2 == c.CF // 2 - 1),
                                     perf_mode=DR)
                mt = pE_t.tile([128, c.NT], BF16, tag="mt")
                nc.scalar.mul(mt[:, 0:n], pd[:, 0:n], 1.0 / (GU_SCALE * WD_SCALE))
                nc.vector.tensor_tensor(h3[mo][:, o:o + n], xo2[mo][:, o:o + n],
                                        mt[:, 0:n], op=AluOpType.add)
            for to in range(o // 128, (o + n) // 128):
                emit_head(to)

        pE_ps.release()
        pF_ps.release()
        pE_t.release()
        pE.release()
        pF_t.release()
        pF.release()
        pE_w.release()
        pD.release()
        p_den.release()
        p_ctxn.release()
        const.release()

    nc.compile()
    return nc


# ===================== host side =====================

def _bf(a):
    return np.ascontiguousarray(np.asarray(a, dtype=np.float32)).astype(NPBF)


def make_tables(c: Cfg):
    pos = np.arange(c.S, dtype=np.float32)
    inv = 1.0 / (c.THETA ** (np.arange(0, c.HD, 2, dtype=np.float32) / c.HD))
    ang = pos[:, None] * inv[None, :]                      # [S, HD/2]
    cos = np.concatenate([np.cos(ang), np.cos(ang)], -1).T  # [HD, S]
    sin = np.concatenate([np.sin(ang), np.sin(ang)], -1).T
    sign = np.where(np.arange(c.HD) < c.HD // 2, -1.0, 1.0)[:, None].astype(np.float32)
    cos_t = _bf(np.tile(cos, (c.HPC, 1)))                  # [128, S]
    sin_t = _bf(np.tile(sin * sign, (c.HPC, 1)))

    shiftT = np.zeros((128, 128), dtype=np.float32)
    for m in range(128):
        src = m + 32 if (m % c.HD) < c.HD // 2 else m - 32
        shiftT[src, m] = 1.0
    sel = np.zeros((c.H, c.D), dtype=np.float32)
    for ch in range(c.CD):
        for m in range(128):
            sel[ch * c.HPC + m // c.HD, ch * 128 + m] = 1.0
    return cos_t, sin_t, _bf(shiftT), _bf(sel)


def tile_lhsT(wT):
    """[K, M] -> [M/128 (mo), 128 (p), K (kc*128+f)] packed lhsT rows.

    out[mo, p, kc*128+f] = wT[kc*128+p, mo*128+f] so one contiguous DMA
    yields the SBUF tile whose [:, kc*128:(kc+1)*128] slice is the
    [K=128, M=128] stationary block for contraction chunk kc.
    """
    K, M = wT.shape
    t = wT.reshape(K // 128, 128, M // 128, 128)       # [kc, p, mo, f]
    return np.ascontiguousarray(t.transpose(2, 1, 0, 3).reshape(M // 128, 128, K))


def tile_fog(wT, W):
    """[D, FF] -> [FF/W (fg), 128 (p), (D/128)*W] packed gate/up slices."""
    D, FF = wT.shape
    t = wT.reshape(D // 128, 128, FF // W, W)          # [kc, p, fg, j]
    return np.ascontiguousarray(
        t.transpose(2, 1, 0, 3).reshape(FF // W, 128, D // 128 * W))


def prep_in_maps(c: Cfg, inputs: dict, n_cores: int = N_CORES):
    x = np.asarray(inputs["chunk_hidden_states"], dtype=np.float32)  # [B,S,D]
    ln1 = np.asarray(inputs["ln1_w"], dtype=np.float32)
    ln2 = np.asarray(inputs["ln2_w"], dtype=np.float32)
    wq = np.asarray(inputs["Wq"], dtype=np.float32)
    wk = np.asarray(inputs["Wk"], dtype=np.float32)
    wv = np.asarray(inputs["Wv"], dtype=np.float32)
    wo = np.asarray(inputs["Wo"], dtype=np.float32)
    wg = np.asarray(inputs["Wgate"], dtype=np.float32)
    wu = np.asarray(inputs["Wup"], dtype=np.float32)
    wd = np.asarray(inputs["Wdown"], dtype=np.float32)
    wout = np.asarray(inputs["W_out"], dtype=np.float32)
    b_out = np.asarray(inputs["b_out"], dtype=np.float32)

    W = min(512, c.FF)
    # q/k/v projection weights in fp8 (x64 for normal-range mantissas; the
    # raw-copy unscales). 1/sqrt(HD) moves into the exp activation scale.
    wqT = tile_lhsT(((wq * ln1[None, :]).T * WD_SCALE).astype(NPF8))
    wkT = tile_lhsT(((wk * ln1[None, :]).T * WD_SCALE).astype(NPF8))
    woT = tile_lhsT(_bf(wo.T))
    wvT = ((wv * ln1[None, :]).T * WD_SCALE).astype(NPF8)
    wgT = tile_fog(_bf((wg * ln2[None, :]).T), W)
    # x32 on the up-weights scales gu into fp8 range; x64 on Wdown keeps it
    # normal-range in fp8. The product 1/2048 is unscaled in the down epilogue.
    wuT = tile_fog(_bf((wu * ln2[None, :] * GU_SCALE).T), W)
    wdT = tile_lhsT((wd.T * WD_SCALE).astype(NPF8))
    woutT = _bf(wout.T)
    bias_row = _bf(b_out[None, :])
    cos_t, sin_t, shiftT, sel = make_tables(c)
    onesb = np.ones((128, 128), dtype=np.float32).astype(NPBF)

    shared = dict(wqT=wqT, wkT=wkT, woT=woT, wvT=wvT, wgT=wgT, wuT=wuT,
                  wdT=wdT, woutT=woutT, bias_row=bias_row, cos_s=cos_t,
                  sin_s=sin_t, shiftT=shiftT, sel=sel, onesb=onesb)

    in_maps = []
    halves = c.S // c.TQ
    for core in range(n_cores):
        b, hf = core // halves, core % halves
        x_fm_f32 = np.ascontiguousarray(x[b].T)                  # [D, S]
        x_fm = x_fm_f32.astype(NPBF)
        x_own = np.ascontiguousarray(x_fm_f32[:, hf * c.TQ:(hf + 1) * c.TQ])
        m = dict(shared)
        m["x_fm"] = x_fm
        m["x_own"] = x_own
        in_maps.append(m)
    return in_maps


_NC_CACHE = {}


def _get_nc(cfg: Cfg):
    if cfg not in _NC_CACHE:
        _NC_CACHE[cfg] = build_bass(cfg)
    return _NC_CACHE[cfg]


def kernel(**inputs) -> np.ndarray:
    c = FULL
    nc = _get_nc(c)
    in_maps = prep_in_maps(c, inputs)
    res = bass_utils.run_bass_kernel_spmd(nc, in_maps, core_ids=list(range(N_CORES)))
    out = np.concatenate([res.results[i]["logits"] for i in range(N_CORES)], axis=0)
    return out.reshape(-1, 8, c.V)
